# revision 15
# baseline (speedup 1.0000x reference)
"""Trainium2 Bass kernel for nn_DynamicRNNEncoder.

Reference semantics (per batch b, steps i = 0..T-1):
    h_prev_i = sum_j conditions[b, i, j] * h_j   (h_j = 0 for j >= i)
    h_i = GRUCell_reset_after(x_i, h_prev_i; kernel, recurrent_kernel, bias)
    out[b, i] = h_i

The graded metric on this setup is wall-clock per call, which is dominated by
the ~35-67 MB/s axon tunnel (h2d ~25ms + size/67MBps per buffer, d2h ~67ms +
size/63MBps), with a ~150ms fixed dispatch floor and ~1ms of actual HW time.
So the design minimizes transferred bytes and buffer count:

  - 2 cores x 32 batches (instead of 8x8): weights are replicated per core,
    so fewer cores = fewer weight bytes on the wire. On-chip cost of the
    bigger per-core batch is microseconds - irrelevant here.
  - ONE packed f32 input tensor per core ("blob", [128 x 15368]): xT | condT |
    wk | wr | bias. One h2d buffer instead of twelve.
  - No shipped eye/ones/zeros/cexp: identity built with affine_select, ones
    with memset, history S zeroed with memset, and h_prev computed from the
    zero-padded history S on device (no host-precomputed scatter operand).
  - Output returned as bf16 (rel err ~2.5e-3 << 2e-2 gate) and converted
    host-side: halves both the donated zero-output upload and the slow fetch.
  - Small program: per-call cost also scales with instruction count (~17ms
    per 1k instructions through lower/serialize/load), so the history matmul
    is chunked (64 matmuls of N=4 per 4 steps instead of 64 of N=1 per step)
    with within-chunk corrections as per-partition tensor_scalar multiplies.
  - jax persistent compilation cache enabled: run_bass_kernel_spmd re-jits a
    fresh closure every call; the cache turns its per-call XLA compile into
    a disk hit (~600ms -> ~50ms).

Precision: all matmuls true fp32 (the recurrence amplifies per-step rounding;
bf16/fp16 anywhere on the inputs fails the 2e-2 gate - measured 0.1-0.16 for
bf16 inputs/weights/conditions, 1.4e-2 for fp16 x). bf16 is only used for the
final output tensor.

Per-core program:
  - Unpack blob -> xT, condT, wk, wr, bias tiles (on-device DMAs).
  - mx = x@wk + bias0' computed in rolling groups of 4 steps (4*32 batch rows
    = 128 partitions); each step reads its mx rows at partition base
    (t%4)*32 in {0,32,64,96}, which is a legal engine access base, so no
    selector matmuls are needed.
  - Every 4 steps: hp4[f,(k,c,b)] = sum_j S[j,(b,c,f)] * condT[j,(t0+k,b)]
    via 64 N=4 matmuls (rows of S at j >= t0 are still zero, matching the
    reference's TensorArray-of-zeros semantics; within-chunk terms are added
    as h_prev += cond[b,t,t0+m]*h_{t0+m} fix terms, per-partition
    tensor_scalar multiplies in B-layout, re-transposed to T-layout).
  - Step t: hpT copy; hpB via PE transpose (+fixes for t%4>0); zr/h
    pre-activations = hpT @ wr (+b1h via rank-1 matmul); gate math on
    [32 x 256] tiles with 1-z = sigmoid(-pre_z); DMA h into S row t.
  - Epilogue: convert S to bf16 and DMA to the output in one shot.
"""

import os
import sys

import numpy as np

for _p in ("/opt/trn_rl_repo", "/root/.axon_site/_ro/trn_rl_repo"):
    if os.path.isdir(_p) and _p not in sys.path:
        sys.path.insert(0, _p)


def _enable_jax_compilation_cache():
    # run_bass_kernel_spmd re-traces and re-compiles its jit on every call
    # (fresh closure); the persistent compilation cache turns the per-call
    # XLA-compile + NEFF-wrapping pipeline into a disk hit (~600ms -> ~50ms).
    try:
        import jax

        jax.config.update("jax_compilation_cache_dir", "/tmp/jax_comp_cache")
        jax.config.update("jax_persistent_cache_min_entry_size_bytes", -1)
        jax.config.update("jax_persistent_cache_min_compile_time_secs", 0.0)
    except Exception:
        pass


_enable_jax_compilation_cache()

B, T, D, H = 64, 128, 256, 256
NCORES = 2
BL = B // NCORES  # 32
H3 = 3 * H

# blob column layout (f32, [128, NCOL])
XT_COLS = 2 * T * BL            # 8192
CT_COLS = T * BL                # 4096
WK_COLS = 2 * H3                # 1536
WR_COLS = 2 * H3                # 1536
BIAS_COLS = 8                   # 1024 floats as [128, 8]
XT0 = 0
CT0 = XT0 + XT_COLS
WK0 = CT0 + CT_COLS
WR0 = WK0 + WK_COLS
BI0 = WR0 + WR_COLS
NCOL = BI0 + BIAS_COLS          # 15368

_CACHE = {}


def _build_program(num_devices=NCORES):
    import concourse.bacc as bacc
    import concourse.mybir as mybir
    import concourse.tile as tile
    from concourse.masks import make_identity

    f32 = mybir.dt.float32
    bf16 = mybir.dt.bfloat16
    ACT = mybir.ActivationFunctionType

    nc = bacc.Bacc("TRN2", target_bir_lowering=False, num_devices=num_devices)

    blob_d = nc.dram_tensor("blob", [128, NCOL], f32, kind="ExternalInput")
    out_d = nc.dram_tensor("out", [T * BL, H], bf16, kind="ExternalOutput")

    with tile.TileContext(nc) as tc:
        with (
            tc.tile_pool(name="consts", bufs=1) as consts,
            tc.tile_pool(name="hist", bufs=1) as hist,
        ):
            xT = consts.tile([128, XT_COLS], f32)
            condT = consts.tile([128, CT_COLS], f32)
            wk = consts.tile([128, WK_COLS], f32)
            wr = consts.tile([128, WR_COLS], f32)
            bias_t = consts.tile([1, 1024], f32)
            eye = consts.tile([128, 128], f32)
            ones = consts.tile([1, 128], f32)

            for t_, c0, cn in (
                (xT, XT0, XT_COLS), (condT, CT0, CT_COLS),
                (wk, WK0, WK_COLS), (wr, WR0, WR_COLS),
            ):
                nc.sync.dma_start(out=t_[:], in_=blob_d.ap()[:, c0:c0 + cn])
            # bias: blob[p, BI0+c] = bias_flat[p*8+c] -> [1, 1024] p-major
            nc.sync.dma_start(
                out=bias_t[:].rearrange("o (p c) -> o p c", p=128),
                in_=blob_d.ap()[:, BI0:BI0 + BIAS_COLS],
            )
            make_identity(nc, eye[:])
            nc.gpsimd.memset(ones[:], 1.0)

            S = hist.tile([128, BL * H], f32)
            nc.vector.memset(S[:], 0.0)
            S_bf = hist.tile([128, BL * H], bf16)

            with (
                tc.tile_pool(name="mxt", bufs=12) as mxt,
                tc.tile_pool(name="promx", bufs=2, space="PSUM") as promx,
                tc.tile_pool(name="php", bufs=2, space="PSUM") as php,
                tc.tile_pool(name="pzr", bufs=1, space="PSUM") as pzr,
                tc.tile_pool(name="pph", bufs=1, space="PSUM") as pph,
                tc.tile_pool(name="phb", bufs=1, space="PSUM") as phb,
                tc.tile_pool(name="phT", bufs=1, space="PSUM") as phT,
                tc.tile_pool(name="work", bufs=2) as work,
                tc.tile_pool(name="hpool", bufs=2) as hpool,
            ):
                mx_tiles = {}

                def do_group(g):
                    # mx for steps 4g..4g+3: computed as [128, 768] in PSUM
                    # (rows (t%4)*32+b), then sliced into per-step base-0
                    # SBUF tiles [32, 768] (engine SBUF reads must start at a
                    # 0/32/64/96 partition base AND DVE two-SBUF-input ops
                    # need equal bases; PSUM reads are exempt, so the slicing
                    # copy is legal at any row offset).
                    step_tiles = [
                        mxt.tile([BL, H3], f32, tag="mxt", name=f"mx{g}_{sl}")
                        for sl in range(4)
                    ]
                    for half in range(2):
                        ps = promx.tile([128, 384], f32, tag="pro")
                        nc.tensor.matmul(
                            ps[:],
                            lhsT=xT[:, g * 128:(g + 1) * 128],
                            rhs=wk[:, half * 384:(half + 1) * 384],
                            start=True, stop=False,
                        )
                        nc.tensor.matmul(
                            ps[:],
                            lhsT=xT[:, T * BL + g * 128: T * BL + (g + 1) * 128],
                            rhs=wk[:, H3 + half * 384: H3 + (half + 1) * 384],
                            start=False, stop=False,
                        )
                        nc.tensor.matmul(
                            ps[:],
                            lhsT=ones[:],
                            rhs=bias_t[:, half * 384:(half + 1) * 384],
                            start=False, stop=True,
                        )
                        for sl in range(4):
                            nc.scalar.copy(
                                step_tiles[sl][:, half * 384:(half + 1) * 384],
                                ps[sl * 32:(sl + 1) * 32, :],
                            )
                    mx_tiles[g] = step_tiles

                do_group(0)
                do_group(1)

                CH = 4  # history-chunk length
                for t in range(T):
                    g, sl = divmod(t, 4)
                    if sl == 0 and g + 2 < T // 4:
                        do_group(g + 2)
                    mxg = mx_tiles[g][sl]
                    q, k = divmod(t, CH)
                    t0 = q * CH

                    if k == 0:
                        # chunk history matmul: hp4[f_lo, k*64 + c*32 + b] =
                        # sum_j S[j,(b,c,f)] * cond[b, t0+k, j] for the 4
                        # steps of this chunk (S rows >= t0 still zero, so
                        # within-chunk terms are added later as fix terms).
                        hp4 = php.tile([128, CH * 2 * BL], f32, tag="hp4")
                        for c in range(2):
                            for b in range(BL):
                                nc.tensor.matmul(
                                    hp4[:].rearrange(
                                        "p (k cb) -> p k cb", k=CH
                                    )[:, :, c * BL + b],
                                    lhsT=S[:, b * H + c * 128:
                                           b * H + (c + 1) * 128],
                                    rhs=condT[:].rearrange(
                                        "p (t b) -> p t b", b=BL
                                    )[:, t0:t0 + CH, b],
                                    start=(c == 0 and b == 0),
                                    stop=(c == 1 and b == BL - 1),
                                    skip_group_check=True,
                                )
                        # cvec_{m,k2}[b] = cond[b, t0+k2, t0+m]: fix-term
                        # coefficients, one [1,32]->[32,1] scatter DMA per
                        # (source step m, target step k2) pair
                        cvecs = {}
                        for m in range(CH - 1):
                            for k2 in range(m + 1, CH):
                                cv = work.tile([BL, 1], f32, tag=f"cv{m}_{k2}")
                                col = (t0 + k2) * BL
                                nc.sync.dma_start(
                                    out=cv[:],
                                    in_=condT[t0 + m: t0 + m + 1,
                                              col: col + BL],
                                )
                                cvecs[(m, k2)] = cv
                        fix_terms = {}
                        chunk_state = (hp4, cvecs, fix_terms)
                    hp4, cvecs, fix_terms = chunk_state

                    hpT_raw = work.tile([128, 2 * BL], f32, tag="hpt")
                    nc.scalar.copy(
                        hpT_raw[:], hp4[:, k * 2 * BL:(k + 1) * 2 * BL]
                    )
                    # B-layout h_prev for the z*h_prev term
                    hpB_raw = phb.tile([BL, H], f32, tag="hpb")
                    for c in range(2):
                        nc.tensor.transpose(
                            hpB_raw[:, c * 128:(c + 1) * 128],
                            hpT_raw[:, c * BL:(c + 1) * BL],
                            eye[:],
                        )
                    if k == 0:
                        hpB = hpB_raw
                        hpT = hpT_raw
                    else:
                        # apply within-chunk history: h_prev += sum_{m<k}
                        # cond[b,t,t0+m] * h_{t0+m}
                        acc = hpB_raw
                        for m in range(k):
                            s = work.tile([BL, H], f32, tag=f"fx{k}_{m}")
                            nc.vector.tensor_add(s[:], acc[:], fix_terms[(m, k)][:])
                            acc = s
                        hpB = acc
                        hpT2 = phT.tile([128, 2 * BL], f32, tag="hpt2")
                        for c in range(2):
                            nc.tensor.transpose(
                                hpT2[:, c * BL:(c + 1) * BL],
                                hpB[:, c * 128:(c + 1) * 128],
                                eye[0:BL, 0:BL],
                            )
                        hpT = work.tile([128, 2 * BL], f32, tag="hptf")
                        nc.scalar.copy(hpT[:], hpT2[:])
                    # pre_zr (recurrent part) = h_prev @ wr_zr
                    zr_ps = pzr.tile([BL, 512], f32, tag="zr")
                    nc.tensor.matmul(
                        zr_ps[:], lhsT=hpT[:, 0:BL], rhs=wr[:, 0:512],
                        start=True, stop=False,
                    )
                    nc.tensor.matmul(
                        zr_ps[:], lhsT=hpT[:, BL:2 * BL],
                        rhs=wr[:, H3: H3 + 512],
                        start=False, stop=True,
                    )
                    # pre_h (recurrent part) = b1h + h_prev @ wr_h
                    ph_ps = pph.tile([BL, H], f32, tag="ph")
                    nc.tensor.matmul(
                        ph_ps[:], lhsT=ones[:, 0:BL], rhs=bias_t[:, H3:1024],
                        start=True, stop=False,
                    )
                    nc.tensor.matmul(
                        ph_ps[:], lhsT=hpT[:, 0:BL], rhs=wr[:, 512:768],
                        start=False, stop=False,
                    )
                    nc.tensor.matmul(
                        ph_ps[:], lhsT=hpT[:, BL:2 * BL],
                        rhs=wr[:, H3 + 512: H3 + 768],
                        start=False, stop=True,
                    )
                    # gates: h = z*hp + (1-z)*cand, 1-z = sigmoid(-pre_z)
                    tzr = work.tile([BL, 512], f32, tag="tzr")
                    nc.vector.tensor_add(
                        tzr[:], zr_ps[:], mxg[:, 0:512]
                    )
                    r_s = work.tile([BL, H], f32, tag="rs")
                    nc.scalar.activation(r_s[:], tzr[:, H:2 * H], ACT.Sigmoid)
                    t1 = work.tile([BL, H], f32, tag="t1")
                    nc.vector.tensor_mul(t1[:], r_s[:], ph_ps[:])
                    z_s = work.tile([BL, H], f32, tag="zs")
                    nc.scalar.activation(z_s[:], tzr[:, 0:H], ACT.Sigmoid)
                    omz = work.tile([BL, H], f32, tag="omz")
                    nc.scalar.activation(
                        omz[:], tzr[:, 0:H], ACT.Sigmoid, scale=-1.0
                    )
                    t2 = work.tile([BL, H], f32, tag="t2")
                    nc.vector.tensor_add(t2[:], t1[:], mxg[:, 512:768])
                    uu = work.tile([BL, H], f32, tag="uu")
                    nc.vector.tensor_mul(uu[:], z_s[:], hpB[:])
                    cand = work.tile([BL, H], f32, tag="cand")
                    nc.scalar.activation(cand[:], t2[:], ACT.Tanh)
                    vv = work.tile([BL, H], f32, tag="vv")
                    nc.vector.tensor_mul(vv[:], omz[:], cand[:])
                    h_s = hpool.tile([BL, H], f32, tag="h")
                    nc.vector.tensor_add(h_s[:], uu[:], vv[:])

                    # fix terms for the remaining steps of this chunk:
                    # term[m->k2] = cond[b, t0+k2, t] * h_t  (per-partition
                    # scalar multiply in B-layout)
                    for k2 in range(k + 1, CH):
                        tm = work.tile([BL, H], f32, tag=f"tm{k}_{k2}")
                        nc.vector.tensor_scalar_mul(
                            tm[:], h_s[:], cvecs[(k, k2)][:]
                        )
                        fix_terms[(k, k2)] = tm

                    nc.sync.dma_start(
                        out=S[t:t + 1, :].rearrange("o (b f) -> o b f", b=BL),
                        in_=h_s[:],
                    )

            # epilogue: S -> bf16 -> out
            for q in range(4):
                nc.vector.tensor_copy(
                    S_bf[:, q * 2048:(q + 1) * 2048],
                    S[:, q * 2048:(q + 1) * 2048],
                )
            nc.sync.dma_start(
                out=out_d.ap().rearrange("(t b) f -> t b f", t=T),
                in_=S_bf[:].rearrange("t (b f) -> t b f", b=BL),
            )

    nc.compile()
    return nc


def _pack_inputs(inputs, conditions, kernel_w, recurrent_kernel, bias):
    """Build the per-core packed input blobs (layout packing only)."""
    wk_p = (
        kernel_w.reshape(2, 128, H3).transpose(1, 0, 2).reshape(128, WK_COLS)
    ).astype(np.float32)
    wr_p = (
        recurrent_kernel.reshape(2, 128, H3).transpose(1, 0, 2).reshape(128, WR_COLS)
    ).astype(np.float32)
    bias0 = bias[0] + np.concatenate([bias[1][: 2 * H], np.zeros(H, np.float32)])
    bias_flat = np.concatenate([bias0, bias[1][2 * H:]]).astype(np.float32)
    bias_pad = bias_flat.reshape(128, BIAS_COLS)

    in_maps = []
    for core in range(NCORES):
        bs = slice(core * BL, (core + 1) * BL)
        x = inputs[bs]  # [BL, T, D]
        # xT[d_lo, c_d*T*BL + t*BL + b] = x[b, t, c_d*128 + d_lo]
        xT = (
            x.reshape(BL, T, 2, 128).transpose(3, 2, 1, 0).reshape(128, XT_COLS)
        )
        cond = conditions[bs]  # [BL, T, T] = [b, i, j]
        # condT[j, t*BL + b] = cond[b, t, j]
        condT = cond.transpose(2, 1, 0).reshape(128, CT_COLS)
        blob = np.concatenate(
            [xT, condT, wk_p, wr_p, bias_pad], axis=1, dtype=np.float32
        )
        in_maps.append({"blob": blob})
    return in_maps


def _run(inputs, conditions, kernel_w, recurrent_kernel, bias, **run_kwargs):
    from concourse.bass_utils import run_bass_kernel_spmd

    if "nc" not in _CACHE:
        _CACHE["nc"] = _build_program()
    nc = _CACHE["nc"]
    in_maps = _pack_inputs(inputs, conditions, kernel_w, recurrent_kernel, bias)
    res = run_bass_kernel_spmd(nc, in_maps, core_ids=list(range(NCORES)), **run_kwargs)
    outs = []
    for core in range(NCORES):
        o = np.asarray(res.results[core]["out"]).astype(np.float32)  # [(t,b), H]
        outs.append(o.reshape(T, BL, H).transpose(1, 0, 2))
    full = np.concatenate(outs, axis=0).astype(np.float32)
    return full, res


def kernel(inputs, conditions, kernel, recurrent_kernel, bias):
    full, _ = _run(
        np.asarray(inputs, np.float32),
        np.asarray(conditions, np.float32),
        np.asarray(kernel, np.float32),
        np.asarray(recurrent_kernel, np.float32),
        np.asarray(bias, np.float32),
    )
    return full
